# revision 36
# baseline (speedup 1.0000x reference)
"""JPEG blocking detector on 8 Trainium2 NeuronCores (Bass/Tile).

Full inputs: tgt (32,3,512,512) f32. Output (32,1,512,512) f32 in {0,1}.
Data-parallel: 4 images per core.

Per image (H=W=512, bs=8, thresh=100):
  lum ~ R + (0.587/0.299) G + (0.114/0.299) B            (scale-invariant)
  e_h = |lum[:, w] - lum[:, w+1]|  -> column sums -> phase bins (w%8)
  e_v = |lum[r, :] - lum[r+1, :]|  -> row sums    -> phase bins (r%8)
  flag_k = psum_k/(counts_k*512) > 100*(total-psum_k)/(other_k*512)
  out[r,w] = maskv[r] OR maskh[w],  maskv[r]=rowflag[r%8]*(r<511), similarly maskh.

Layout: partition p holds CONSECUTIVE image rows 4p..4p+3 (free dim = (k,w)).
  - host pre-scales channels by luma weights, casts to bf16 (halves HBM read)
  - vertical diffs are free-dim shifts in a partition; boundary rows from one
    PE matmul pair (S*lum0 - I'*lum3); |dh| on the scalar engine
  - row sums via scalar Abs+accumulator / vector reduce; column sums via
    ones-column PE matmuls into a pair-shared PSUM strip (N=511, cols 511
    pre-zeroed once)
  - flag algebra batched per image-PAIR on one partition:
      flag  <=>  (-cB)*tot < ph*(cA-cB)   (eps dropped: strict compare)
  - output fp8 (0/1 exact), upcast on host
"""

import numpy as np
from contextlib import ExitStack

import ml_dtypes

NCORES = 8
NB = 4          # images per core
P = 128         # partitions
K4 = 4          # rows per partition
W = 512
C1 = 0.587 / 0.299
C2 = 0.114 / 0.299

IN_NPDT = ml_dtypes.bfloat16
OUT_NPDT = ml_dtypes.float8_e4m3

# balance knobs: dv row-sum segments on vector (rest scalar); output k's on
# vector ts-max (rest scalar activation), per image: front images keep vector
# free for later stats, tail images split across both engines
DV_V_KS = (0,)
OUT_V_KS_PER_B = {0: (1, 2, 3), 1: (1, 2, 3), 2: (0, 1), 3: (0, 1)}


def _make_consts():
    # bf16 block (128 x 257): [S | negI' | ones_col]
    cb = np.zeros((128, 257), np.float32)
    for m in range(127):
        cb[m + 1, m] = 1.0
        cb[m, 128 + m] = -1.0
    cb[:, 256] = 1.0
    CBS = cb.astype(ml_dtypes.bfloat16)

    # bf16 row block (1 x 512): [ones128 | evenind | oddind | odd127z]
    cd = np.zeros((1, 512), np.float32)
    cd[0, 0:128] = 1.0
    cd[0, 128:256] = (np.arange(128) % 2 == 0).astype(np.float32)
    cd[0, 256:384] = (np.arange(128) % 2 == 1).astype(np.float32)
    cd[0, 384:512] = cd[0, 256:384]
    cd[0, 511] = 0.0  # odd127z: excludes (p=127,k=3) i.e. image row 511
    CD = cd.astype(ml_dtypes.bfloat16)

    # f32 col block (128 x 2): [Eev | Eod] for row-phase matmuls (rhs s is f32)
    ce = np.zeros((128, 2), np.float32)
    ce[:, 0] = (np.arange(128) % 2 == 0).astype(np.float32)
    ce[:, 1] = (np.arange(128) % 2 == 1).astype(np.float32)

    # f32 row block (1 x 64): [cAB32 | negcB32]  (per-pair flag algebra)
    counts = np.array([64] * 7 + [63], np.float32)
    other = 511.0 - counts
    cA8 = 1.0 / (counts * 512.0)
    cB8 = -100.0 / (other * 512.0)
    cA16 = np.concatenate([cA8, cA8])
    cB16 = np.concatenate([cB8, cB8])
    cf = np.zeros((1, 64), np.float32)
    cf[0, 0:32] = np.concatenate([cA16 - cB16, cA16 - cB16])
    cf[0, 32:64] = np.concatenate([-cB16, -cB16])
    return CBS, CD, ce, cf


def _kernel_body(ctx, tc, out, x, cbs, cd, ce, cf):
    import concourse.bass as bass  # noqa: F401
    from concourse import mybir
    from concourse.alu_op_type import AluOpType as alu

    nc = tc.nc
    f32 = mybir.dt.float32
    bf16 = mybir.dt.bfloat16
    fp8 = mybir.dt.float8e4
    Abs = mybir.ActivationFunctionType.Abs
    Ident = mybir.ActivationFunctionType.Identity
    Copy = mybir.ActivationFunctionType.Copy
    X = mybir.AxisListType.X

    singles = ctx.enter_context(tc.tile_pool(name="singles", bufs=1))
    pin = ctx.enter_context(tc.tile_pool(name="pin", bufs=4))
    pwork = ctx.enter_context(tc.tile_pool(name="pwork", bufs=4))
    posb = ctx.enter_context(tc.tile_pool(name="posb", bufs=3))
    ptiny = ctx.enter_context(tc.tile_pool(name="ptiny", bufs=4))
    ppair = ctx.enter_context(tc.tile_pool(name="ppair", bufs=2))
    pbnd = ctx.enter_context(tc.tile_pool(name="pbnd", bufs=2, space="PSUM"))
    ppsc = ctx.enter_context(tc.tile_pool(name="ppsc", bufs=1, space="PSUM"))
    pmh = ctx.enter_context(tc.tile_pool(name="pmh", bufs=1, space="PSUM"))
    ptp = ctx.enter_context(tc.tile_pool(name="ptp", bufs=2, space="PSUM"))

    csb = singles.tile([128, 257], bf16, tag="csb")
    nc.gpsimd.dma_start(out=csb, in_=cbs)
    cds = singles.tile([1, 512], bf16, tag="cds")
    nc.gpsimd.dma_start(out=cds, in_=cd)
    cse = singles.tile([128, 2], f32, tag="cse")
    nc.gpsimd.dma_start(out=cse, in_=ce)
    csf = singles.tile([1, 64], f32, tag="csf")
    nc.gpsimd.dma_start(out=csf, in_=cf)

    Smat = csb[:, 0:128]
    negI = csb[:, 128:256]
    ones_col = csb[:, 256:257]
    ones_row = cds[0:1, 0:128]
    even_row = cds[0:1, 128:256]
    odd_row = cds[0:1, 256:384]
    odd127z = cds[0:1, 384:512]
    Eev = cse[:, 0:1]
    Eod = cse[:, 1:2]
    cAB32 = csf[0:1, 0:32]
    negcB32 = csf[0:1, 32:64]

    # pair-shared PSUM column-sum strip; cols 511 of each image never written
    # by the N=511 matmuls -> zero once
    psc = ppsc.tile([1, 2, 512], f32, tag="psc")
    nc.vector.memset(psc[:, :, 511:512], 0.0)

    # mh PSUM tiles: col 511 never written (N=504 + N=7 matmuls) -> zero both
    mh_tiles = []
    for mi in range(2):
        mh0 = pmh.tile([P, W], f32, tag=f"mh{mi}")
        nc.vector.memset(mh0[:, 511:512], 0.0)
        mh_tiles.append(mh0)

    def stats_phase(b, ph32):
        pj = b % 2
        rgb = pin.tile([P, 3, K4, W], bf16, tag="rgb")
        nc.sync.dma_start(out=rgb, in_=x[b].rearrange("c (p k) w -> p c k w", p=P))

        t1 = pwork.tile([P, K4, W], bf16, tag="t1")
        nc.vector.tensor_tensor(t1, rgb[:, 0], rgb[:, 1], alu.add)
        lum = pwork.tile([P, K4, W], bf16, tag="lum")
        nc.vector.tensor_tensor(lum, t1, rgb[:, 2], alu.add)

        # vertical: in-partition diffs + PE boundary pair (emitted first so
        # the scalar-engine row-sum accumulators start as early as possible)
        bnd = pbnd.tile([P, W], f32, tag="bnd")
        nc.tensor.matmul(bnd, lhsT=Smat, rhs=lum[:, 0], start=True, stop=False)
        nc.tensor.matmul(bnd, lhsT=negI, rhs=lum[:, 3], start=False, stop=True)
        dvt = pwork.tile([P, 3, W], bf16, tag="dvt")
        nc.vector.tensor_tensor(dvt, lum[:, 1:4], lum[:, 0:3], alu.subtract)

        # horizontal |dh| -> PE column sums into psc strip (cols 0..510)
        dha = pwork.tile([P, K4, W], bf16, tag="dha")
        nc.vector.tensor_tensor(
            dha[:, :, 0:511], lum[:, :, 1:512], lum[:, :, 0:511], alu.subtract
        )
        adh = pwork.tile([P, K4, W], bf16, tag="adh")
        nc.scalar.activation(adh[:, :, 0:511], dha[:, :, 0:511], Abs)
        for k in range(K4):
            nc.tensor.matmul(
                psc[0:1, pj, 0:511], lhsT=ones_col, rhs=adh[:, k, 0:511],
                start=(k == 0), stop=(k == 3),
            )

        s = ptiny.tile([P, K4], f32, tag="s")
        scrap = pwork.tile([P, K4, W], bf16, tag="scrap")
        for k in range(3):
            if k in DV_V_KS:
                nc.vector.tensor_reduce(
                    s[:, k : k + 1], dvt[:, k : k + 1], axis=X, op=alu.add,
                    apply_absolute_value=True,
                )
            else:
                nc.scalar.activation(
                    scrap[:, k], dvt[:, k], Abs, accum_out=s[:, k : k + 1]
                )
        nc.scalar.activation(scrap[:, 3], bnd, Abs, accum_out=s[:, 3:4])

        # row-phase partials: red2 = [even(4) | odd(4)] partition sums of s
        tp = ptp.tile([P, 12], f32, tag="tp")
        red2 = tp[0:1, 0:8]
        nc.tensor.matmul(red2[0:1, 0:4], lhsT=Eev, rhs=s, start=True, stop=True)
        nc.tensor.matmul(red2[0:1, 4:8], lhsT=Eod, rhs=s, start=True, stop=True)
        # rows: phase j = k + 4*(p%2): [ev k=0..3 -> ph 0..3 | od -> ph 4..7]
        nc.scalar.copy(ph32[0:1, 16 * pj + 8 : 16 * pj + 16], red2)
        return tp

    def flags_phase(ph32):
        # column phases: fold psc strip (both images) into ph32[0:8]/[16:24]
        nc.vector.tensor_reduce(
            ph32.rearrange("p (a x) -> p a x", a=2)[:, :, 0:8],
            psc.rearrange("p a (i j) -> p a j i", j=8),
            axis=X, op=alu.add,
        )
        tot4 = ppair.tile([1, 4], f32, tag="tot4")
        nc.vector.tensor_reduce(
            tot4, ph32.rearrange("p (g j) -> p g j", j=8), axis=X, op=alu.add
        )
        q = ppair.tile([1, 32], f32, tag="q")
        nc.vector.tensor_tensor(q, ph32, cAB32, alu.mult)
        v2 = ppair.tile([1, 32], f32, tag="v2")
        tot_b = tot4.unsqueeze(2).broadcast_to([1, 4, 8])
        nc.vector.tensor_tensor(
            v2.rearrange("p (g j) -> p g j", j=8), negcB32.rearrange("p (g j) -> p g j", j=8),
            tot_b, alu.mult,
        )
        flagsP = ppair.tile([1, 32], f32, tag="flagsP")
        nc.vector.tensor_tensor(flagsP, v2, q, alu.is_lt)
        flags16 = ppair.tile([1, 32], bf16, tag="flags16")
        nc.scalar.copy(flags16, flagsP)
        return flags16

    def out_phase(b, flags16, tp):
        pj = b % 2
        fr = flags16[0:1, 16 * pj : 16 * pj + 16]
        # maskv[p,k] = rowflag[k+4*(p%2)], with (127,3) zeroed via odd127z
        mvp = tp[:, 8:12]
        nc.tensor.matmul(mvp[:, 0:3], lhsT=even_row, rhs=fr[0:1, 8:11], start=True, stop=False)
        nc.tensor.matmul(mvp[:, 0:3], lhsT=odd_row, rhs=fr[0:1, 12:15], start=False, stop=True)
        nc.tensor.matmul(mvp[:, 3:4], lhsT=even_row, rhs=fr[0:1, 11:12], start=True, stop=False)
        nc.tensor.matmul(mvp[:, 3:4], lhsT=odd127z, rhs=fr[0:1, 15:16], start=False, stop=True)
        OUT_V_KS = OUT_V_KS_PER_B[b]
        if len(OUT_V_KS) < 4:
            mv = ptiny.tile([P, K4], f32, tag="mv")
            nc.scalar.copy(mv, mvp)
            nmv = ptiny.tile([P, K4], f32, tag="nmv")
            nc.scalar.activation(nmv, mvp, Copy, bias=1.0, scale=-1.0)

        # maskh replicated to all partitions; col 511 stays 0 (pre-zeroed)
        mh = mh_tiles[b % 2]
        bc = fr[0:1, 0:8].unsqueeze(1)
        nc.tensor.matmul(
            mh[:, 0:504], lhsT=ones_row, rhs=bc.broadcast_to([1, 63, 8]),
            start=True, stop=True,
        )
        nc.tensor.matmul(
            mh[:, 504:511], lhsT=ones_row, rhs=fr[0:1, 0:7],
            start=True, stop=True,
        )

        osb = posb.tile([P, K4, W], fp8, tag="osb")
        for k in range(K4):
            if k in OUT_V_KS:
                nc.vector.tensor_scalar(
                    osb[:, k], mh, mvp[:, k : k + 1], None, alu.max
                )
            else:
                nc.scalar.activation(
                    osb[:, k], mh, Ident,
                    bias=mv[:, k : k + 1], scale=nmv[:, k : k + 1],
                )
        odeng = nc.sync if b == 3 else nc.gpsimd
        odeng.dma_start(
            out=out[b, 0].rearrange("(p k) w -> p k w", p=P), in_=osb
        )

    for pi in range(2):
        ph32 = ppair.tile([1, 32], f32, tag="ph32")
        tps = [stats_phase(2 * pi + j, ph32) for j in range(2)]
        flags16 = flags_phase(ph32)
        for j in range(2):
            out_phase(2 * pi + j, flags16, tps[j])


_CACHED_NC = None


def _build_nc():
    global _CACHED_NC
    if _CACHED_NC is not None:
        return _CACHED_NC
    import concourse.bass as bass
    import concourse.tile as tile
    from concourse import bacc, mybir

    nc = bacc.Bacc("TRN2", target_bir_lowering=False, debug=False)
    x = nc.dram_tensor("x", [NB, 3, 512, 512], mybir.dt.bfloat16, kind="ExternalInput").ap()
    cbs = nc.dram_tensor("cbs", [128, 257], mybir.dt.bfloat16, kind="ExternalInput").ap()
    cd = nc.dram_tensor("cd", [1, 512], mybir.dt.bfloat16, kind="ExternalInput").ap()
    ce = nc.dram_tensor("ce", [128, 2], mybir.dt.float32, kind="ExternalInput").ap()
    cf = nc.dram_tensor("cf", [1, 64], mybir.dt.float32, kind="ExternalInput").ap()
    out = nc.dram_tensor(
        "out", [NB, 1, 512, 512], mybir.dt.float8e4, kind="ExternalOutput"
    ).ap()
    with tile.TileContext(nc) as tc, ExitStack() as ctx:
        _kernel_body(ctx, tc, out, x, cbs, cd, ce, cf)
    if not nc.is_finalized():
        nc.finalize()
    _CACHED_NC = nc
    return nc


def make_in_maps(tgt):
    CBS, CD, CE, CF = _make_consts()
    tgt32 = np.asarray(tgt, dtype=np.float32)
    wch = np.array([1.0, C1, C2], np.float32).reshape(1, 3, 1, 1)
    tgt16 = (tgt32 * wch).astype(IN_NPDT)
    return [
        {"x": tgt16[i * NB : (i + 1) * NB], "cbs": CBS, "cd": CD, "ce": CE, "cf": CF}
        for i in range(NCORES)
    ]


def run(tgt, **kwargs):
    from concourse.bass_utils import run_bass_kernel_spmd

    nc = _build_nc()
    res = run_bass_kernel_spmd(nc, make_in_maps(tgt), core_ids=list(range(NCORES)), **kwargs)
    full = np.concatenate([r["out"] for r in res.results], axis=0).astype(np.float32)
    return full, res


def kernel(tgt):
    full, _ = run(tgt)
    return full


# revision 37
# speedup vs baseline: 1.0258x; 1.0258x over previous
"""JPEG blocking detector on 8 Trainium2 NeuronCores (Bass/Tile).

Full inputs: tgt (32,3,512,512) f32. Output (32,1,512,512) f32 in {0,1}.
Data-parallel: 4 images per core.

Per image (H=W=512, bs=8, thresh=100):
  lum ~ R + (0.587/0.299) G + (0.114/0.299) B            (scale-invariant)
  e_h = |lum[:, w] - lum[:, w+1]|  -> column sums -> phase bins (w%8)
  e_v = |lum[r, :] - lum[r+1, :]|  -> row sums    -> phase bins (r%8)
  flag_k = psum_k/(counts_k*512) > 100*(total-psum_k)/(other_k*512)
  out[r,w] = maskv[r] OR maskh[w],  maskv[r]=rowflag[r%8]*(r<511), similarly maskh.

Layout: partition p holds CONSECUTIVE image rows 4p..4p+3 (free dim = (k,w)).
  - host pre-scales channels by luma weights, casts to bf16 (halves HBM read)
  - vertical diffs are free-dim shifts in a partition; boundary rows from one
    PE matmul pair (S*lum0 - I'*lum3); |dh| on the scalar engine
  - row sums via scalar Abs+accumulator / vector reduce; column sums via
    ones-column PE matmuls into a pair-shared PSUM strip (N=511, cols 511
    pre-zeroed once)
  - flag algebra batched per image-PAIR on one partition:
      flag  <=>  (-cB)*tot < ph*(cA-cB)   (eps dropped: strict compare)
  - output fp8 (0/1 exact), upcast on host
"""

import numpy as np
from contextlib import ExitStack

import ml_dtypes

NCORES = 8
NB = 4          # images per core
P = 128         # partitions
K4 = 4          # rows per partition
W = 512
C1 = 0.587 / 0.299
C2 = 0.114 / 0.299

IN_NPDT = ml_dtypes.bfloat16
OUT_NPDT = ml_dtypes.float8_e4m3

# balance knobs: dv row-sum segments on vector (rest scalar); output k's on
# vector ts-max (rest scalar activation), per image: front images keep vector
# free for later stats, tail images split across both engines
DV_V_KS = (0,)
OUT_V_KS_PER_B = {0: (1, 2, 3), 1: (1, 2, 3), 2: (0, 1), 3: (0, 1)}


def _make_consts():
    # bf16 block (128 x 257): [S | negI' | ones_col]
    cb = np.zeros((128, 257), np.float32)
    for m in range(127):
        cb[m + 1, m] = 1.0
        cb[m, 128 + m] = -1.0
    cb[:, 256] = 1.0
    CBS = cb.astype(ml_dtypes.bfloat16)

    # bf16 row block (1 x 512): [ones128 | evenind | oddind | odd127z]
    cd = np.zeros((1, 512), np.float32)
    cd[0, 0:128] = 1.0
    cd[0, 128:256] = (np.arange(128) % 2 == 0).astype(np.float32)
    cd[0, 256:384] = (np.arange(128) % 2 == 1).astype(np.float32)
    cd[0, 384:512] = cd[0, 256:384]
    cd[0, 511] = 0.0  # odd127z: excludes (p=127,k=3) i.e. image row 511
    CD = cd.astype(ml_dtypes.bfloat16)

    # f32 col block (128 x 2): [Eev | Eod] for row-phase matmuls (rhs s is f32)
    ce = np.zeros((128, 2), np.float32)
    ce[:, 0] = (np.arange(128) % 2 == 0).astype(np.float32)
    ce[:, 1] = (np.arange(128) % 2 == 1).astype(np.float32)

    # f32 row block (1 x 64): [cAB32 | negcB32]  (per-pair flag algebra)
    counts = np.array([64] * 7 + [63], np.float32)
    other = 511.0 - counts
    cA8 = 1.0 / (counts * 512.0)
    cB8 = -100.0 / (other * 512.0)
    cA16 = np.concatenate([cA8, cA8])
    cB16 = np.concatenate([cB8, cB8])
    cf = np.zeros((1, 64), np.float32)
    cf[0, 0:32] = np.concatenate([cA16 - cB16, cA16 - cB16])
    cf[0, 32:64] = np.concatenate([-cB16, -cB16])
    return CBS, CD, ce, cf


def _kernel_body(ctx, tc, out, x, cbs, cd, ce, cf):
    import concourse.bass as bass  # noqa: F401
    from concourse import mybir
    from concourse.alu_op_type import AluOpType as alu

    nc = tc.nc
    f32 = mybir.dt.float32
    bf16 = mybir.dt.bfloat16
    fp8 = mybir.dt.float8e4
    Abs = mybir.ActivationFunctionType.Abs
    Ident = mybir.ActivationFunctionType.Identity
    Copy = mybir.ActivationFunctionType.Copy
    X = mybir.AxisListType.X

    singles = ctx.enter_context(tc.tile_pool(name="singles", bufs=1))
    pin = ctx.enter_context(tc.tile_pool(name="pin", bufs=4))
    pwork = ctx.enter_context(tc.tile_pool(name="pwork", bufs=4))
    posb = ctx.enter_context(tc.tile_pool(name="posb", bufs=3))
    ptiny = ctx.enter_context(tc.tile_pool(name="ptiny", bufs=4))
    ppair = ctx.enter_context(tc.tile_pool(name="ppair", bufs=2))
    pbnd = ctx.enter_context(tc.tile_pool(name="pbnd", bufs=2, space="PSUM"))
    ppsc = ctx.enter_context(tc.tile_pool(name="ppsc", bufs=1, space="PSUM"))
    pmh = ctx.enter_context(tc.tile_pool(name="pmh", bufs=1, space="PSUM"))
    ptp = ctx.enter_context(tc.tile_pool(name="ptp", bufs=2, space="PSUM"))

    csb = singles.tile([128, 257], bf16, tag="csb")
    nc.gpsimd.dma_start(out=csb, in_=cbs)
    cds = singles.tile([1, 512], bf16, tag="cds")
    nc.gpsimd.dma_start(out=cds, in_=cd)
    cse = singles.tile([128, 2], f32, tag="cse")
    nc.gpsimd.dma_start(out=cse, in_=ce)
    csf = singles.tile([1, 64], f32, tag="csf")
    nc.gpsimd.dma_start(out=csf, in_=cf)

    Smat = csb[:, 0:128]
    negI = csb[:, 128:256]
    ones_col = csb[:, 256:257]
    ones_row = cds[0:1, 0:128]
    even_row = cds[0:1, 128:256]
    odd_row = cds[0:1, 256:384]
    odd127z = cds[0:1, 384:512]
    Eev = cse[:, 0:1]
    Eod = cse[:, 1:2]
    cAB32 = csf[0:1, 0:32]
    negcB32 = csf[0:1, 32:64]

    # pair-shared PSUM column-sum strip; cols 511 of each image never written
    # by the N=511 matmuls -> zero once
    psc = ppsc.tile([1, 2, 512], f32, tag="psc")
    nc.vector.memset(psc[:, :, 511:512], 0.0)

    # mh PSUM tiles: col 511 never written (N=504 + N=7 matmuls) -> zero both
    mh_tiles = []
    for mi in range(2):
        mh0 = pmh.tile([P, W], f32, tag=f"mh{mi}")
        nc.vector.memset(mh0[:, 511:512], 0.0)
        mh_tiles.append(mh0)

    def stats_phase(b, ph32):
        pj = b % 2
        rgb = pin.tile([P, 3, K4, W], bf16, tag="rgb")
        nc.sync.dma_start(out=rgb, in_=x[b].rearrange("c (p k) w -> p c k w", p=P))

        t1 = pwork.tile([P, K4, W], bf16, tag="t1")
        nc.vector.tensor_tensor(t1, rgb[:, 0], rgb[:, 1], alu.add)
        lum = pwork.tile([P, K4, W], bf16, tag="lum")
        nc.vector.tensor_tensor(lum, t1, rgb[:, 2], alu.add)

        # horizontal |dh| -> PE column sums into psc strip (cols 0..510)
        dha = pwork.tile([P, K4, W], bf16, tag="dha")
        nc.vector.tensor_tensor(
            dha[:, :, 0:511], lum[:, :, 1:512], lum[:, :, 0:511], alu.subtract
        )
        adh = pwork.tile([P, K4, W], bf16, tag="adh")
        nc.scalar.activation(adh[:, :, 0:511], dha[:, :, 0:511], Abs)
        for k in range(K4):
            nc.tensor.matmul(
                psc[0:1, pj, 0:511], lhsT=ones_col, rhs=adh[:, k, 0:511],
                start=(k == 0), stop=(k == 3),
            )

        # vertical: in-partition diffs + PE boundary pair
        bnd = pbnd.tile([P, W], f32, tag="bnd")
        nc.tensor.matmul(bnd, lhsT=Smat, rhs=lum[:, 0], start=True, stop=False)
        nc.tensor.matmul(bnd, lhsT=negI, rhs=lum[:, 3], start=False, stop=True)
        dvt = pwork.tile([P, 3, W], bf16, tag="dvt")
        nc.vector.tensor_tensor(dvt, lum[:, 1:4], lum[:, 0:3], alu.subtract)

        s = ptiny.tile([P, K4], f32, tag="s")
        scrap = pwork.tile([P, K4, W], bf16, tag="scrap")
        for k in range(3):
            if k in DV_V_KS:
                nc.vector.tensor_reduce(
                    s[:, k : k + 1], dvt[:, k : k + 1], axis=X, op=alu.add,
                    apply_absolute_value=True,
                )
            else:
                nc.scalar.activation(
                    scrap[:, k], dvt[:, k], Abs, accum_out=s[:, k : k + 1]
                )
        nc.scalar.activation(scrap[:, 3], bnd, Abs, accum_out=s[:, 3:4])

        # row-phase partials: red2 = [even(4) | odd(4)] partition sums of s
        tp = ptp.tile([P, 12], f32, tag="tp")
        red2 = tp[0:1, 0:8]
        nc.tensor.matmul(red2[0:1, 0:4], lhsT=Eev, rhs=s, start=True, stop=True)
        nc.tensor.matmul(red2[0:1, 4:8], lhsT=Eod, rhs=s, start=True, stop=True)
        # rows: phase j = k + 4*(p%2): [ev k=0..3 -> ph 0..3 | od -> ph 4..7]
        nc.scalar.copy(ph32[0:1, 16 * pj + 8 : 16 * pj + 16], red2)
        return tp

    def flags_phase(ph32):
        # column phases: fold psc strip (both images) into ph32[0:8]/[16:24]
        nc.vector.tensor_reduce(
            ph32.rearrange("p (a x) -> p a x", a=2)[:, :, 0:8],
            psc.rearrange("p a (i j) -> p a j i", j=8),
            axis=X, op=alu.add,
        )
        tot4 = ppair.tile([1, 4], f32, tag="tot4")
        nc.vector.tensor_reduce(
            tot4, ph32.rearrange("p (g j) -> p g j", j=8), axis=X, op=alu.add
        )
        q = ppair.tile([1, 32], f32, tag="q")
        nc.vector.tensor_tensor(q, ph32, cAB32, alu.mult)
        v2 = ppair.tile([1, 32], f32, tag="v2")
        tot_b = tot4.unsqueeze(2).broadcast_to([1, 4, 8])
        nc.vector.tensor_tensor(
            v2.rearrange("p (g j) -> p g j", j=8), negcB32.rearrange("p (g j) -> p g j", j=8),
            tot_b, alu.mult,
        )
        flagsP = ppair.tile([1, 32], f32, tag="flagsP")
        nc.vector.tensor_tensor(flagsP, v2, q, alu.is_lt)
        flags16 = ppair.tile([1, 32], bf16, tag="flags16")
        nc.scalar.copy(flags16, flagsP)
        return flags16

    def out_phase(b, flags16, tp):
        pj = b % 2
        fr = flags16[0:1, 16 * pj : 16 * pj + 16]
        # maskv[p,k] = rowflag[k+4*(p%2)], with (127,3) zeroed via odd127z
        mvp = tp[:, 8:12]
        nc.tensor.matmul(mvp[:, 0:3], lhsT=even_row, rhs=fr[0:1, 8:11], start=True, stop=False)
        nc.tensor.matmul(mvp[:, 0:3], lhsT=odd_row, rhs=fr[0:1, 12:15], start=False, stop=True)
        nc.tensor.matmul(mvp[:, 3:4], lhsT=even_row, rhs=fr[0:1, 11:12], start=True, stop=False)
        nc.tensor.matmul(mvp[:, 3:4], lhsT=odd127z, rhs=fr[0:1, 15:16], start=False, stop=True)
        OUT_V_KS = OUT_V_KS_PER_B[b]
        if len(OUT_V_KS) < 4:
            mv = ptiny.tile([P, K4], f32, tag="mv")
            nc.scalar.copy(mv, mvp)
            nmv = ptiny.tile([P, K4], f32, tag="nmv")
            nc.scalar.activation(nmv, mvp, Copy, bias=1.0, scale=-1.0)

        # maskh replicated to all partitions; col 511 stays 0 (pre-zeroed)
        mh = mh_tiles[b % 2]
        bc = fr[0:1, 0:8].unsqueeze(1)
        nc.tensor.matmul(
            mh[:, 0:504], lhsT=ones_row, rhs=bc.broadcast_to([1, 63, 8]),
            start=True, stop=True,
        )
        nc.tensor.matmul(
            mh[:, 504:511], lhsT=ones_row, rhs=fr[0:1, 0:7],
            start=True, stop=True,
        )

        osb = posb.tile([P, K4, W], fp8, tag="osb")
        for k in range(K4):
            if k in OUT_V_KS:
                nc.vector.tensor_scalar(
                    osb[:, k], mh, mvp[:, k : k + 1], None, alu.max
                )
            else:
                nc.scalar.activation(
                    osb[:, k], mh, Ident,
                    bias=mv[:, k : k + 1], scale=nmv[:, k : k + 1],
                )
        odeng = nc.sync if b == 3 else nc.gpsimd
        odeng.dma_start(
            out=out[b, 0].rearrange("(p k) w -> p k w", p=P), in_=osb
        )

    for pi in range(2):
        ph32 = ppair.tile([1, 32], f32, tag="ph32")
        tps = [stats_phase(2 * pi + j, ph32) for j in range(2)]
        flags16 = flags_phase(ph32)
        for j in range(2):
            out_phase(2 * pi + j, flags16, tps[j])


_CACHED_NC = None


def _build_nc():
    global _CACHED_NC
    if _CACHED_NC is not None:
        return _CACHED_NC
    import concourse.bass as bass
    import concourse.tile as tile
    from concourse import bacc, mybir

    nc = bacc.Bacc("TRN2", target_bir_lowering=False, debug=False)
    x = nc.dram_tensor("x", [NB, 3, 512, 512], mybir.dt.bfloat16, kind="ExternalInput").ap()
    cbs = nc.dram_tensor("cbs", [128, 257], mybir.dt.bfloat16, kind="ExternalInput").ap()
    cd = nc.dram_tensor("cd", [1, 512], mybir.dt.bfloat16, kind="ExternalInput").ap()
    ce = nc.dram_tensor("ce", [128, 2], mybir.dt.float32, kind="ExternalInput").ap()
    cf = nc.dram_tensor("cf", [1, 64], mybir.dt.float32, kind="ExternalInput").ap()
    out = nc.dram_tensor(
        "out", [NB, 1, 512, 512], mybir.dt.float8e4, kind="ExternalOutput"
    ).ap()
    with tile.TileContext(nc) as tc, ExitStack() as ctx:
        _kernel_body(ctx, tc, out, x, cbs, cd, ce, cf)
    if not nc.is_finalized():
        nc.finalize()
    _CACHED_NC = nc
    return nc


def make_in_maps(tgt):
    CBS, CD, CE, CF = _make_consts()
    tgt32 = np.asarray(tgt, dtype=np.float32)
    wch = np.array([1.0, C1, C2], np.float32).reshape(1, 3, 1, 1)
    tgt16 = (tgt32 * wch).astype(IN_NPDT)
    return [
        {"x": tgt16[i * NB : (i + 1) * NB], "cbs": CBS, "cd": CD, "ce": CE, "cf": CF}
        for i in range(NCORES)
    ]


def run(tgt, **kwargs):
    from concourse.bass_utils import run_bass_kernel_spmd

    nc = _build_nc()
    res = run_bass_kernel_spmd(nc, make_in_maps(tgt), core_ids=list(range(NCORES)), **kwargs)
    full = np.concatenate([r["out"] for r in res.results], axis=0).astype(np.float32)
    return full, res


def kernel(tgt):
    full, _ = run(tgt)
    return full
